# revision 1
# baseline (speedup 1.0000x reference)
"""BracketNet Trainium2 kernel, v3: time-sliced scan chains, fused cat matmul.

Time is split into T chains with burn-in (the recurrence is contractive:
divergence from a wrong initial ctx decays below 1e-6 within ~22 steps, so
each chain restarts from ctx=0 a MIN_BURN window early). Each joint-step is ONE
K=128 matmul over a shared cat^T tile ([128, J*64]: partitions 0:64 = ctx^T
written by the gelu ACT, 64:128 = x^T written by the input DMA), halving PE
work (one LDWEIGHTS+MATMUL instead of two). The r-add reads x^T from a
base-0 copy made once per chunk by a single wide DVE copy (cross-partition
single-input copies are legal; two-input ops require equal base partitions).

Everything 2-byte fp16 (x, ctx, weights, r): full-rate PE, DVE 2x/4x modes,
halved DMA; fp32 PSUM accumulation and fp32 ACT internals keep error ~3e-4.
"""

import numpy as np

S, B, D, H = 2048, 64, 512, 8
DIM = 64

J = 8            # chains per joint group (one ACT instruction covers J)
P = 3            # joint groups (latency-hiding factor)
T = J * P        # total time-sliced chains
L = 102          # steps each chain executes (own + burn-in)
CH = 6           # joint-steps per streamed chunk
MIN_BURN = 16    # minimum burn-in steps for chains 1..T-1

REPS = 1         # repeat the whole body (timing runs only)

JB = J * B       # free width of one joint-step block (J chains x 64 batch)

_last_run_info = {}


def _chain_ends():
    rest = S - L
    base, extra = divmod(rest, T - 1)
    assert base + 1 <= L - MIN_BURN, "burn-in too short; raise L or lower T"
    owns = [L] + [base + 1] * extra + [base] * (T - 1 - extra)
    ends = np.cumsum(owns)
    assert ends[-1] == S
    return [int(e) for e in ends], owns


def _build_nc():
    import concourse.mybir as mybir
    from concourse import tile, bacc

    f32 = mybir.dt.float32
    f16 = mybir.dt.float16
    nc = bacc.Bacc("TRN2", target_bir_lowering=False, debug=False)

    NSTREAM = P * L * JB
    xt_ext = nc.declare_dram_parameter("xt", [DIM, NSTREAM], f16, isOutput=False)
    wt_ext = nc.declare_dram_parameter("wt", [2 * DIM, DIM], f16, isOutput=False)
    bias_ext = nc.declare_dram_parameter("bias", [DIM, 1], f32, isOutput=False)
    rt_ext = nc.declare_dram_parameter("rt", [DIM, NSTREAM], f16, isOutput=True)

    gelu = mybir.ActivationFunctionType.Gelu
    NCH = L // CH
    assert NCH * CH == L

    with tile.TileContext(nc) as tc:
        with (
            tc.tile_pool(name="const", bufs=1) as cpool,
            tc.tile_pool(name="catp", bufs=3) as catpool,
            tc.tile_pool(name="xlp", bufs=2) as xlpool,
            tc.tile_pool(name="rst", bufs=2) as rpool,
            tc.tile_pool(name="ps", bufs=2, space="PSUM") as ppool,
        ):
            wt = cpool.tile([2 * DIM, DIM], f16, tag="wt", name="wt")
            nc.sync.dma_start(out=wt[:], in_=wt_ext[:])
            bias = cpool.tile([DIM, 1], f32, tag="bias", name="bias")
            nc.sync.dma_start(out=bias[:], in_=bias_ext[:])

            def body():
                def new_cat(p):
                    return catpool.tile(
                        [2 * DIM, CH * JB], f16, tag=f"cat{p}", name=f"cat{p}"
                    )

                def dma_x(dest_tiles, c):
                    for p in range(P):
                        lo = (p * L + c * CH) * JB
                        hi = (p * L + (c + 1) * CH) * JB
                        nc.gpsimd.dma_start(
                            out=dest_tiles[p][DIM : 2 * DIM, :],
                            in_=xt_ext[:, lo:hi],
                        )

                cat = [new_cat(p) for p in range(P)]
                dma_x(cat, 0)
                for p in range(P):
                    nc.vector.memset(cat[p][0:DIM, 0:JB], 0.0)  # ctx_{-1} = 0

                for c in range(NCH):
                    cat_next = [new_cat(p) for p in range(P)]
                    if c + 1 < NCH:
                        dma_x(cat_next, c + 1)
                    xlo = [
                        xlpool.tile(
                            [DIM, CH * JB], f16, tag=f"xlo{p}", name=f"xlo{p}"
                        )
                        for p in range(P)
                    ]
                    for p in range(P):
                        # base-0 copy of x^T for the r-add (single wide copy)
                        nc.vector.tensor_copy(xlo[p][:], cat[p][DIM : 2 * DIM, :])
                    r = [
                        rpool.tile([DIM, CH * JB], f16, tag=f"r{p}", name=f"r{p}")
                        for p in range(P)
                    ]
                    for i in range(CH):
                        for p in range(P):
                            sl = slice(i * JB, (i + 1) * JB)
                            ps = ppool.tile(
                                [DIM, JB], f32, tag=f"y{p}", name=f"y{p}"
                            )
                            nc.tensor.matmul(
                                ps[:], wt[:], cat[p][:, sl], start=True, stop=True
                            )
                            if i + 1 < CH:
                                dest = cat[p][0:DIM, (i + 1) * JB : (i + 2) * JB]
                            else:
                                dest = cat_next[p][0:DIM, 0:JB]
                            nc.scalar.activation(dest, ps[:], gelu, bias=bias[:])
                            nc.vector.tensor_add(r[p][:, sl], xlo[p][:, sl], dest)
                    for p in range(P):
                        lo = (p * L + c * CH) * JB
                        hi = (p * L + (c + 1) * CH) * JB
                        nc.sync.dma_start(out=rt_ext[:, lo:hi], in_=r[p][:])
                    cat = cat_next

            if REPS == 1:
                body()
            else:
                with tc.For_i(0, REPS, 1):
                    body()

    nc.compile()
    return nc


_nc_cache = None


def _get_nc():
    global _nc_cache
    if _nc_cache is None:
        _nc_cache = _build_nc()
    return _nc_cache


def _make_in_maps(src, W, b):
    ends, owns = _chain_ends()
    in_maps = []
    for h in range(H):
        xh = src[:, :, h * DIM : (h + 1) * DIM]  # [S, B, DIM] (t, b, d)
        xt = np.empty((DIM, P, L, J, B), dtype=np.float32)
        for k in range(T):
            p, j = divmod(k, J)
            s0 = ends[k] - L
            xt[:, p, :, j, :] = xh[s0 : s0 + L].transpose(2, 0, 1)
        in_maps.append(
            {
                "xt": np.ascontiguousarray(xt)
                .reshape(DIM, P * L * JB)
                .astype(np.float16),
                "wt": np.ascontiguousarray(W[h].T).astype(np.float16),
                "bias": np.ascontiguousarray(b[h].reshape(DIM, 1)),
            }
        )
    return in_maps, ends, owns


def _assemble(results, ends, owns):
    out = np.empty((S, B, D), dtype=np.float32)
    for h in range(H):
        rt = results[h]["rt"].astype(np.float32).reshape(DIM, P, L, J, B)
        for k in range(T):
            p, j = divmod(k, J)
            own = owns[k]
            blk = rt[:, p, L - own :, j, :]  # [DIM, own, B]
            out[ends[k] - own : ends[k], :, h * DIM : (h + 1) * DIM] = (
                blk.transpose(1, 2, 0)
            )
    return out


def kernel(src: np.ndarray, W: np.ndarray, b: np.ndarray) -> np.ndarray:
    import os
    from concourse.bass_utils import run_bass_kernel_spmd

    src = np.ascontiguousarray(src, dtype=np.float32)
    W = np.asarray(W, dtype=np.float32)
    b = np.asarray(b, dtype=np.float32)

    nc = _get_nc()
    in_maps, ends, owns = _make_in_maps(src, W, b)

    trace = bool(os.environ.get("BASS_TRACE"))
    res = run_bass_kernel_spmd(nc, in_maps, list(range(H)), trace=trace)
    _last_run_info["exec_time_ns"] = res.exec_time_ns
    _last_run_info["profile_json"] = res.profile_json

    return _assemble(res.results, ends, owns)



# revision 2
# speedup vs baseline: 1.4132x; 1.4132x over previous
"""BracketNet Trainium2 kernel, v4: partition-paired chain groups.

Two chain half-groups (J chains x B batch each) stack in the partition dim:
ctx lives in a standalone [128, J*B] tile (half-group A on partitions 0:64,
B on 64:128) and x in a matching [128, CH*J*B] streamed tile. Each joint
step is TWO matmuls accumulating into one PSUM bank — x-projection with a
block-diagonal Wx (start) and ctx-projection with block-diagonal Wc (stop) —
followed by ONE [128, J*B] gelu and ONE [128, J*B] add. Relative to v3 this
halves ACT and DVE instruction counts (the two previous bottleneck engines,
86%/80% busy) and eliminates the per-chunk x^T base-0 copies entirely (x,
ctx, r all share partition layout, so the r-add is directly legal). PE
streaming work is unchanged (2x N=512 matmuls per pair-step == 1 per group
step before).

Time-slicing as v3: T chains with contractive burn-in (error from ctx=0
restart decays ~0.53/step), PG=2 supergroups rotate to hide the serial
matmul->gelu->matmul latency. Everything fp16 except PSUM/ACT internals.
"""

import numpy as np

S, B, D, H = 2048, 64, 512, 8
DIM = 64

J = 8            # chains per half-group (matmul free width = J*B = 512)
PG = 2           # supergroups rotating for latency hiding
T = 2 * J * PG   # total time-sliced chains (two half-groups per supergroup)
L = 80           # steps each chain executes (own + burn-in)
CH = 8           # joint-steps per streamed chunk
MIN_BURN = 16    # minimum burn-in steps for chains 1..T-1

N = J * B        # free width of one pair-step block

_last_run_info = {}


def _chain_ends():
    rest = S - L
    base, extra = divmod(rest, T - 1)
    assert base + 1 <= L - MIN_BURN, "burn-in too short; raise L or lower T"
    owns = [L] + [base + 1] * extra + [base] * (T - 1 - extra)
    ends = np.cumsum(owns)
    assert ends[-1] == S
    return [int(e) for e in ends], owns


def _build_nc():
    import concourse.mybir as mybir
    from concourse import tile, bacc

    f32 = mybir.dt.float32
    f16 = mybir.dt.float16
    nc = bacc.Bacc("TRN2", target_bir_lowering=False, debug=False)

    NSTREAM = PG * L * N
    xt_ext = nc.declare_dram_parameter("xt", [2 * DIM, NSTREAM], f16, isOutput=False)
    wc_ext = nc.declare_dram_parameter("wc", [2 * DIM, 2 * DIM], f16, isOutput=False)
    wx_ext = nc.declare_dram_parameter("wx", [2 * DIM, 2 * DIM], f16, isOutput=False)
    bias_ext = nc.declare_dram_parameter("bias", [2 * DIM, 1], f32, isOutput=False)
    rt_ext = nc.declare_dram_parameter("rt", [2 * DIM, NSTREAM], f16, isOutput=True)

    gelu = mybir.ActivationFunctionType.Gelu
    NCH = L // CH
    assert NCH * CH == L

    with tile.TileContext(nc) as tc:
        with (
            tc.tile_pool(name="const", bufs=1) as cpool,
            tc.tile_pool(name="xp", bufs=2) as xpool,
            tc.tile_pool(name="cx", bufs=3) as cxpool,
            tc.tile_pool(name="rst", bufs=2) as rpool,
            tc.tile_pool(name="ps", bufs=2, space="PSUM") as ppool,
        ):
            wc = cpool.tile([2 * DIM, 2 * DIM], f16, tag="wc", name="wc")
            nc.sync.dma_start(out=wc[:], in_=wc_ext[:])
            wx = cpool.tile([2 * DIM, 2 * DIM], f16, tag="wx", name="wx")
            nc.sync.dma_start(out=wx[:], in_=wx_ext[:])
            bias = cpool.tile([2 * DIM, 1], f32, tag="bias", name="bias")
            nc.sync.dma_start(out=bias[:], in_=bias_ext[:])

            def new_x(s):
                return xpool.tile([2 * DIM, CH * N], f16, tag=f"x{s}", name=f"x{s}")

            def dma_x(dest_tiles, c):
                for s in range(PG):
                    lo = (s * L + c * CH) * N
                    hi = (s * L + (c + 1) * CH) * N
                    nc.gpsimd.dma_start(out=dest_tiles[s][:], in_=xt_ext[:, lo:hi])

            xcur = [new_x(s) for s in range(PG)]
            dma_x(xcur, 0)
            ctx = [None] * PG

            for c in range(NCH):
                xnext = [new_x(s) for s in range(PG)]
                if c + 1 < NCH:
                    dma_x(xnext, c + 1)
                r = [
                    rpool.tile([2 * DIM, CH * N], f16, tag=f"r{s}", name=f"r{s}")
                    for s in range(PG)
                ]
                for i in range(CH):
                    sl = slice(i * N, (i + 1) * N)
                    ps = []
                    for s in range(PG):
                        p = ppool.tile([2 * DIM, N], f32, tag=f"ps{s}", name=f"ps{s}")
                        ps.append(p)
                        first = c == 0 and i == 0
                        nc.tensor.matmul(
                            p[:], wx[:], xcur[s][:, sl], start=True, stop=first
                        )
                        if not first:
                            nc.tensor.matmul(
                                p[:], wc[:], ctx[s][:], start=False, stop=True
                            )
                    for s in range(PG):
                        cnew = cxpool.tile(
                            [2 * DIM, N], f16, tag=f"c{s}", name=f"c{s}"
                        )
                        nc.scalar.activation(cnew[:], ps[s][:], gelu, bias=bias[:])
                        ctx[s] = cnew
                    for s in range(PG):
                        nc.vector.tensor_add(r[s][:, sl], xcur[s][:, sl], ctx[s][:])
                for s in range(PG):
                    lo = (s * L + c * CH) * N
                    hi = (s * L + (c + 1) * CH) * N
                    nc.sync.dma_start(out=rt_ext[:, lo:hi], in_=r[s][:])
                xcur = xnext

    nc.compile()
    return nc


_nc_cache = None


def _get_nc():
    global _nc_cache
    if _nc_cache is None:
        _nc_cache = _build_nc()
    return _nc_cache


def _make_in_maps(src, W, b):
    ends, owns = _chain_ends()
    in_maps = []
    for h in range(H):
        xh = src[:, :, h * DIM : (h + 1) * DIM]  # [S, B, DIM] (t, b, d)
        xt = np.empty((2 * DIM, PG, L, J, B), dtype=np.float32)
        for k in range(T):
            s, rem = divmod(k, 2 * J)
            half, j = divmod(rem, J)
            t0 = ends[k] - L
            xt[half * DIM : (half + 1) * DIM, s, :, j, :] = xh[t0 : t0 + L].transpose(
                2, 0, 1
            )
        wc_t = np.ascontiguousarray(W[h][:, :DIM].T)  # [64 in, 64 out]
        wx_t = np.ascontiguousarray(W[h][:, DIM:].T)
        z = np.zeros((DIM, DIM), dtype=np.float32)
        wc2 = np.block([[wc_t, z], [z, wc_t]])
        wx2 = np.block([[wx_t, z], [z, wx_t]])
        in_maps.append(
            {
                "xt": np.ascontiguousarray(xt)
                .reshape(2 * DIM, PG * L * N)
                .astype(np.float16),
                "wc": wc2.astype(np.float16),
                "wx": wx2.astype(np.float16),
                "bias": np.ascontiguousarray(
                    np.concatenate([b[h], b[h]]).reshape(2 * DIM, 1)
                ),
            }
        )
    return in_maps, ends, owns


def _assemble(results, ends, owns):
    out = np.empty((S, B, D), dtype=np.float32)
    for h in range(H):
        rt = results[h]["rt"].astype(np.float32).reshape(2 * DIM, PG, L, J, B)
        for k in range(T):
            s, rem = divmod(k, 2 * J)
            half, j = divmod(rem, J)
            own = owns[k]
            blk = rt[half * DIM : (half + 1) * DIM, s, L - own :, j, :]
            out[ends[k] - own : ends[k], :, h * DIM : (h + 1) * DIM] = blk.transpose(
                1, 2, 0
            )
    return out


def kernel(src: np.ndarray, W: np.ndarray, b: np.ndarray) -> np.ndarray:
    import os
    from concourse.bass_utils import run_bass_kernel_spmd

    src = np.ascontiguousarray(src, dtype=np.float32)
    W = np.asarray(W, dtype=np.float32)
    b = np.asarray(b, dtype=np.float32)

    nc = _get_nc()
    in_maps, ends, owns = _make_in_maps(src, W, b)

    trace = bool(os.environ.get("BASS_TRACE"))
    res = run_bass_kernel_spmd(nc, in_maps, list(range(H)), trace=trace)
    _last_run_info["exec_time_ns"] = res.exec_time_ns
    _last_run_info["profile_json"] = res.profile_json

    return _assemble(res.results, ends, owns)


# revision 4
# speedup vs baseline: 1.6933x; 1.1982x over previous
"""BracketNet Trainium2 kernel, v4: partition-paired chain groups.

Two chain half-groups (J chains x B batch each) stack in the partition dim:
ctx lives in a standalone [128, J*B] tile (half-group A on partitions 0:64,
B on 64:128) and x in a matching [128, CH*J*B] streamed tile. Each joint
step is TWO matmuls accumulating into one PSUM bank — x-projection with a
block-diagonal Wx (start) and ctx-projection with block-diagonal Wc (stop) —
followed by ONE [128, J*B] gelu and ONE [128, J*B] add. Relative to v3 this
halves ACT and DVE instruction counts (the two previous bottleneck engines,
86%/80% busy) and eliminates the per-chunk x^T base-0 copies entirely (x,
ctx, r all share partition layout, so the r-add is directly legal). PE
streaming work is unchanged (2x N=512 matmuls per pair-step == 1 per group
step before).

Time-slicing as v3: T chains with contractive burn-in (error from ctx=0
restart decays ~0.53/step), PG=2 supergroups rotate to hide the serial
matmul->gelu->matmul latency. Everything fp16 except PSUM/ACT internals.
"""

import numpy as np

S, B, D, H = 2048, 64, 512, 8
DIM = 64

J = 8            # chains per half-group (matmul free width = J*B = 512)
PG = 2           # supergroups rotating for latency hiding
T = 2 * J * PG   # total time-sliced chains (two half-groups per supergroup)
L = 72           # steps each chain executes (own + burn-in)
MIN_BURN = 8     # minimum burn-in steps for chains 1..T-1

# chunk schedule: small chunks at the edges so the first x DMA (which gates
# compute start) and the last r DMA (the tail) are tiny; big in steady state
CHUNKS = [2, 2, 4, 8, 8, 8, 8, 8, 8, 8, 4, 2, 2]
assert sum(CHUNKS) == L

N = J * B        # free width of one pair-step block

_last_run_info = {}


def _chain_ends():
    rest = S - L
    base, extra = divmod(rest, T - 1)
    assert base + 1 <= L - MIN_BURN, "burn-in too short; raise L or lower T"
    owns = [L] + [base + 1] * extra + [base] * (T - 1 - extra)
    ends = np.cumsum(owns)
    assert ends[-1] == S
    return [int(e) for e in ends], owns


def _build_nc():
    import concourse.mybir as mybir
    from concourse import tile, bacc

    f32 = mybir.dt.float32
    f16 = mybir.dt.float16
    nc = bacc.Bacc("TRN2", target_bir_lowering=False, debug=False)

    NSTREAM = PG * L * N
    xt_ext = nc.declare_dram_parameter("xt", [2 * DIM, NSTREAM], f16, isOutput=False)
    wc_ext = nc.declare_dram_parameter("wc", [2 * DIM, 2 * DIM], f16, isOutput=False)
    wx_ext = nc.declare_dram_parameter("wx", [2 * DIM, 2 * DIM], f16, isOutput=False)
    bias_ext = nc.declare_dram_parameter("bias", [2 * DIM, 1], f32, isOutput=False)
    rt_ext = nc.declare_dram_parameter("rt", [2 * DIM, NSTREAM], f16, isOutput=True)

    gelu = mybir.ActivationFunctionType.Gelu
    NCH = len(CHUNKS)
    OFFS = [sum(CHUNKS[:c]) for c in range(NCH)]

    with tile.TileContext(nc) as tc:
        with (
            tc.tile_pool(name="const", bufs=1) as cpool,
            tc.tile_pool(name="xp", bufs=3) as xpool,
            tc.tile_pool(name="cx", bufs=3) as cxpool,
            tc.tile_pool(name="rst", bufs=3) as rpool,
            tc.tile_pool(name="ps", bufs=2, space="PSUM") as ppool,
        ):
            wc = cpool.tile([2 * DIM, 2 * DIM], f16, tag="wc", name="wc")
            nc.sync.dma_start(out=wc[:], in_=wc_ext[:])
            wx = cpool.tile([2 * DIM, 2 * DIM], f16, tag="wx", name="wx")
            nc.sync.dma_start(out=wx[:], in_=wx_ext[:])
            bias = cpool.tile([2 * DIM, 1], f32, tag="bias", name="bias")
            nc.sync.dma_start(out=bias[:], in_=bias_ext[:])

            xtiles = {}

            def ensure_x(c):
                if c >= NCH or c in xtiles:
                    return
                sz = CHUNKS[c]
                tiles = [
                    xpool.tile([2 * DIM, sz * N], f16, tag=f"x{s}", name=f"x{s}")
                    for s in range(PG)
                ]
                for s in range(PG):
                    lo = (s * L + OFFS[c]) * N
                    hi = (s * L + OFFS[c] + sz) * N
                    nc.gpsimd.dma_start(out=tiles[s][:], in_=xt_ext[:, lo:hi])
                xtiles[c] = tiles

            ensure_x(0)
            ensure_x(1)
            ensure_x(2)
            ctx = [None] * PG

            for c in range(NCH):
                sz = CHUNKS[c]
                xcur = xtiles.pop(c)
                ensure_x(c + 2)
                r = [
                    rpool.tile([2 * DIM, sz * N], f16, tag=f"r{s}", name=f"r{s}")
                    for s in range(PG)
                ]
                for i in range(sz):
                    sl = slice(i * N, (i + 1) * N)
                    ps = []
                    for s in range(PG):
                        p = ppool.tile([2 * DIM, N], f32, tag=f"ps{s}", name=f"ps{s}")
                        ps.append(p)
                        first = c == 0 and i == 0
                        nc.tensor.matmul(
                            p[:], wx[:], xcur[s][:, sl], start=True, stop=first
                        )
                        if not first:
                            nc.tensor.matmul(
                                p[:], wc[:], ctx[s][:], start=False, stop=True
                            )
                    for s in range(PG):
                        cnew = cxpool.tile(
                            [2 * DIM, N], f16, tag=f"c{s}", name=f"c{s}"
                        )
                        nc.scalar.activation(cnew[:], ps[s][:], gelu, bias=bias[:])
                        ctx[s] = cnew
                    for s in range(PG):
                        nc.vector.tensor_add(r[s][:, sl], xcur[s][:, sl], ctx[s][:])
                for s in range(PG):
                    lo = (s * L + OFFS[c]) * N
                    hi = (s * L + OFFS[c] + sz) * N
                    nc.sync.dma_start(out=rt_ext[:, lo:hi], in_=r[s][:])

    nc.compile()
    return nc


_nc_cache = None


def _get_nc():
    global _nc_cache
    if _nc_cache is None:
        _nc_cache = _build_nc()
    return _nc_cache


def _make_in_maps(src, W, b):
    ends, owns = _chain_ends()
    in_maps = []
    for h in range(H):
        xh = src[:, :, h * DIM : (h + 1) * DIM]  # [S, B, DIM] (t, b, d)
        xt = np.empty((2 * DIM, PG, L, J, B), dtype=np.float32)
        for k in range(T):
            s, rem = divmod(k, 2 * J)
            half, j = divmod(rem, J)
            t0 = ends[k] - L
            xt[half * DIM : (half + 1) * DIM, s, :, j, :] = xh[t0 : t0 + L].transpose(
                2, 0, 1
            )
        wc_t = np.ascontiguousarray(W[h][:, :DIM].T)  # [64 in, 64 out]
        wx_t = np.ascontiguousarray(W[h][:, DIM:].T)
        z = np.zeros((DIM, DIM), dtype=np.float32)
        wc2 = np.block([[wc_t, z], [z, wc_t]])
        wx2 = np.block([[wx_t, z], [z, wx_t]])
        in_maps.append(
            {
                "xt": np.ascontiguousarray(xt)
                .reshape(2 * DIM, PG * L * N)
                .astype(np.float16),
                "wc": wc2.astype(np.float16),
                "wx": wx2.astype(np.float16),
                "bias": np.ascontiguousarray(
                    np.concatenate([b[h], b[h]]).reshape(2 * DIM, 1)
                ),
            }
        )
    return in_maps, ends, owns


def _assemble(results, ends, owns):
    out = np.empty((S, B, D), dtype=np.float32)
    for h in range(H):
        rt = results[h]["rt"].astype(np.float32).reshape(2 * DIM, PG, L, J, B)
        for k in range(T):
            s, rem = divmod(k, 2 * J)
            half, j = divmod(rem, J)
            own = owns[k]
            blk = rt[half * DIM : (half + 1) * DIM, s, L - own :, j, :]
            out[ends[k] - own : ends[k], :, h * DIM : (h + 1) * DIM] = blk.transpose(
                1, 2, 0
            )
    return out


def kernel(src: np.ndarray, W: np.ndarray, b: np.ndarray) -> np.ndarray:
    import os
    from concourse.bass_utils import run_bass_kernel_spmd

    src = np.ascontiguousarray(src, dtype=np.float32)
    W = np.asarray(W, dtype=np.float32)
    b = np.asarray(b, dtype=np.float32)

    nc = _get_nc()
    in_maps, ends, owns = _make_in_maps(src, W, b)

    trace = bool(os.environ.get("BASS_TRACE"))
    res = run_bass_kernel_spmd(nc, in_maps, list(range(H)), trace=trace)
    _last_run_info["exec_time_ns"] = res.exec_time_ns
    _last_run_info["profile_json"] = res.profile_json

    return _assemble(res.results, ends, owns)
